# revision 24
# baseline (speedup 1.0000x reference)
"""Causal multi-head attention (B=2, S=2048, D=2048, H=16, DH=128) on 8 TRN2
NeuronCores.

Sharding: data-parallel over batch (2) x tensor-parallel over heads (4 groups
of 4 heads). Core c handles batch c//4, heads 4*(c%4) .. 4*(c%4)+3. Each core
computes its heads' attention and a partial output projection; the host sums
the 4 partials per batch (the "all-reduce") and applies the 2^-12 descale.

Numerics / speed scheme (validated to rel err ~1.7e-3 in emulation):
  - QKV projection: 3-term fp8e4 hi/lo DoubleRow matmuls (K=256 per instr,
    0.5 cycles/row -> 0.75x the f32r cycle count). Host pre-quantizes
    16*x and 256*w into packed pair layouts [D/256, 128, 2, *]; the 2^-12
    descale is applied at the PSUM->SBUF copy. Q,K,V stored bf16.
  - scores: bf16 matmuls (1 cycle/row, no N>=256 floor so diagonal blocks
    trim at 128 granularity), f32 PSUM.
  - softmax: exp on ACT with scale 1/sqrt(dh), bias -10 (scores bounded),
    es in bf16. Causal masking: 128-wide triangular mask multiply on the
    first valid 128 columns of diagonal blocks only.
  - denominators: es-as-stationary matmuls (out [128q, 1] per 128-q chunk,
    ~free on the PE), transposed back to a [1, 512] strip via 4 single-row
    PE transposes, reciprocal on DVE, gpsimd partition broadcast. The ones
    vector is 1/16 so bc = 16/den and O is produced at 16x scale for fp8.
  - PV: bf16 matmuls accumulating in PSUM.
  - output projection: 3-term fp8e4 hi/lo DoubleRow over E-pairs; O
    quantized to fp8 hi/lo pair tiles at 16x scale on DVE; host divides
    the summed partials by 4096 (= 16*256).
Phases are interleaved: attention for q-chunk qc runs right after the
projection chunk sc=qc (it needs exactly K/V chunks 0..qc), and the output
projection for qc-1 is spread through the attention of qc to fill PE stalls.
"""

import sys

if "/opt/trn_rl_repo" not in sys.path:
    sys.path.insert(0, "/opt/trn_rl_repo")

import numpy as np
import ml_dtypes

import concourse.bass as bass  # noqa: F401  (registers AP types)
import concourse.tile as tile
from concourse import bacc, mybir
from concourse.bass_utils import run_bass_kernel_spmd

B, S, D = 2, 2048, 2048
H, DH = 16, 128
HL = H // 4          # heads per core
E = HL * DH          # local feature width (512)
SCALE = 1.0 / np.sqrt(DH)
CBIAS = 10.0         # > max causal score (8.70 measured on the real inputs)

F32 = mybir.dt.float32
F32R = mybir.dt.float32r
F8 = mybir.dt.float8e4
BF = mybir.dt.bfloat16
DR = mybir.MatmulPerfMode.DoubleRow
E4NP = ml_dtypes.float8_e4m3
BFNP = ml_dtypes.bfloat16

NSC = S // 512       # s-chunks of 512
NDP = D // 256       # D-pair groups (8)
DESCALE = 2.0 ** -12  # 1/(16*256)
WARMUP = 16          # PE warm-up matmuls bridging the initial DMA window


def build_program(s=S):
    nsc = s // 512
    nc = bacc.Bacc("TRN2", target_bir_lowering=False, debug=False, num_devices=8)

    xh8 = nc.dram_tensor("xh8", [NDP, 128, 2, s], F8, kind="ExternalInput").ap()
    xl8 = nc.dram_tensor("xl8", [NDP, 128, 2, s], F8, kind="ExternalInput").ap()
    w8 = {}
    for wn in ("q", "k", "v"):
        for part in ("h", "l"):
            name = f"w{wn}{part}8"
            w8[wn, part] = nc.dram_tensor(name, [NDP, 128, 2, E], F8,
                                          kind="ExternalInput").ap()
    woh8 = nc.dram_tensor("woh8", [2, 128, 2, D], F8, kind="ExternalInput").ap()
    wol8 = nc.dram_tensor("wol8", [2, 128, 2, D], F8, kind="ExternalInput").ap()
    eye = nc.dram_tensor("eye", [128, 128], F32R, kind="ExternalInput").ap()
    mask = nc.dram_tensor("mask", [128, 128], BF, kind="ExternalInput").ap()
    ones16 = nc.dram_tensor("ones16", [128, 1], BF, kind="ExternalInput").ap()
    out_part = nc.dram_tensor("out_part", [s, D], BF, kind="ExternalOutput").ap()

    with tile.TileContext(nc) as tc:
        _emit(tc, nc, xh8, xl8, w8, woh8, wol8, eye, mask, ones16, out_part, nsc)
    nc.compile()
    return nc


def _emit(tc, nc, xh8, xl8, w8, woh8, wol8, eye, mask, ones16, out_part, nsc):
    from contextlib import ExitStack
    ctx = ExitStack()
    s = nsc * 512

    # ---- constants / long-lived tiles -----------------------------------
    const_pool = ctx.enter_context(tc.tile_pool(name="const", bufs=1))
    bias_t = const_pool.tile([128, 1], F32, tag="bias", name="bias")
    nc.vector.memset(bias_t[:], -CBIAS)
    eye_t = const_pool.tile([128, 128], F32R, tag="eye", name="eye")
    mask_t = const_pool.tile([128, 128], BF, tag="mask", name="mask")
    ones_t = const_pool.tile([128, 1], BF, tag="ones", name="ones")

    # ---- PE warm-up during the initial DMA ramp -------------------------
    with tc.tile_pool(name="warm", bufs=1) as warm_pool, \
         tc.tile_pool(name="warmps", bufs=1, space="PSUM") as warm_ps:
        wsrc = warm_pool.tile([128, 512], F32, tag="wsrc", name="wsrc")
        nc.vector.memset(wsrc[:], 0.0)
        wps = warm_ps.tile([128, 512], F32, tag="wps", name="wps")
        for _ in range(WARMUP):
            nc.tensor.matmul(wps[:, :256], wsrc[:, :128], wsrc[:, :256],
                             start=True, stop=True)

    # ---- persistent products --------------------------------------------
    qk_pool = ctx.enter_context(tc.tile_pool(name="qk", bufs=1))
    QT = [qk_pool.tile([128, s], BF, tag=f"qT{h}", name=f"qT{h}") for h in range(HL)]
    KT = [qk_pool.tile([128, s], BF, tag=f"kT{h}", name=f"kT{h}") for h in range(HL)]
    v_pool = ctx.enter_context(tc.tile_pool(name="v", bufs=1))
    V = [v_pool.tile([128, E], BF, tag=f"v{kt}", name=f"v{kt}")
         for kt in range(4 * nsc)]

    # ---- weights ---------------------------------------------------------
    w_pool = ctx.enter_context(tc.tile_pool(name="w", bufs=1))
    wt = {}
    for wn in ("q", "k", "v"):
        for part in ("h", "l"):
            wt[wn, part] = [
                w_pool.tile([128, 2, E], F8, tag=f"w{wn}{part}{dp}",
                            name=f"w{wn}{part}{dp}") for dp in range(NDP)]
    wo_pool = ctx.enter_context(tc.tile_pool(name="wo", bufs=1))
    wo_t = {}
    for part, dram in (("h", woh8), ("l", wol8)):
        wo_t[part] = [wo_pool.tile([128, 2, D], F8, tag=f"wo{part}{t}",
                                   name=f"wo{part}{t}") for t in range(2)]

    # O pair tiles (fp8 hi/lo, 16x scale), per head-pair t, double-buffered
    # across qc (outproj of qc-1 runs during attention of qc).
    o_pool = ctx.enter_context(tc.tile_pool(name="o", bufs=2))

    # ---- phase-2 pools ---------------------------------------------------
    es_pool = ctx.enter_context(tc.tile_pool(name="es", bufs=6))
    nrm_pool = ctx.enter_context(tc.tile_pool(name="nrm", bufs=2))
    res_pool = ctx.enter_context(tc.tile_pool(name="res", bufs=4))
    x_pool = ctx.enter_context(tc.tile_pool(name="x", bufs=2))

    pa = ctx.enter_context(tc.tile_pool(name="pa", bufs=2, space="PSUM"))
    sp = ctx.enter_context(tc.tile_pool(name="sp", bufs=2, space="PSUM"))
    op = ctx.enter_context(tc.tile_pool(name="op", bufs=2, space="PSUM"))
    dnp = ctx.enter_context(tc.tile_pool(name="dnp", bufs=1, space="PSUM"))
    ptp = ctx.enter_context(tc.tile_pool(name="ptp", bufs=1, space="PSUM"))

    # ---- initial DMAs ----------------------------------------------------
    def load_x_chunk(sc):
        xt = {}
        for dp in range(NDP):
            for part, dram in (("h", xh8), ("l", xl8)):
                t = x_pool.tile([128, 2, 512], F8, tag=f"x{part}{dp}",
                                name=f"x{part}{dp}")
                nc.sync.dma_start(
                    t[:], dram[dp, :, :, sc * 512:(sc + 1) * 512])
                xt[part, dp] = t
        return xt

    # DMA priority, matched to the term-major chain order (hh steps first):
    # wq_hi + x_hi unblock the first 8 chain steps after only 2MB, then
    # wq_lo (hl terms), x_lo (lh terms), then wk, wv.
    x_cur = {}
    for dp in range(NDP):
        nc.sync.dma_start(wt["q", "h"][dp][:], w8["q", "h"][dp])
        t = x_pool.tile([128, 2, 512], F8, tag=f"xh{dp}", name=f"xh{dp}")
        nc.sync.dma_start(t[:], xh8[dp, :, :, 0:512])
        x_cur["h", dp] = t
    for dp in range(NDP):
        nc.sync.dma_start(wt["q", "l"][dp][:], w8["q", "l"][dp])
    for dp in range(NDP):
        t = x_pool.tile([128, 2, 512], F8, tag=f"xl{dp}", name=f"xl{dp}")
        nc.sync.dma_start(t[:], xl8[dp, :, :, 0:512])
        x_cur["l", dp] = t
    for dp in range(NDP):
        nc.sync.dma_start(wt["k", "h"][dp][:], w8["k", "h"][dp])
        nc.sync.dma_start(wt["k", "l"][dp][:], w8["k", "l"][dp])
    for dp in range(NDP):
        nc.sync.dma_start(wt["v", "h"][dp][:], w8["v", "h"][dp])
        nc.sync.dma_start(wt["v", "l"][dp][:], w8["v", "l"][dp])
    nc.sync.dma_start(eye_t[:], eye)
    nc.sync.dma_start(mask_t[:], mask)
    nc.sync.dma_start(ones_t[:], ones16)
    for part in ("h", "l"):
        for t in range(2):
            nc.sync.dma_start(wo_t[part][t][:], (woh8 if part == "h" else wol8)[t])

    # pending output-projection units: (qc, j, dc) tuples
    pending_units = []
    o_tiles = {}   # qc -> {("h"|"l", t): tile}

    def emit_unit():
        if not pending_units:
            return
        qc, j, dc = pending_units.pop(0)
        ot = o_tiles[qc]
        jsl = slice(j * 128, (j + 1) * 128)
        dsl = slice(dc * 512, (dc + 1) * 512)
        ps_f = pa.tile([128, 512], F32, tag="pa", name="pf")
        steps = []
        for t in range(2):
            steps += [(ot["h", t], wo_t["h"][t]), (ot["h", t], wo_t["l"][t]),
                      (ot["l", t], wo_t["h"][t])]
        for i, (o8, w8t) in enumerate(steps):
            nc.tensor.matmul(ps_f[:], o8[:, :, jsl], w8t[:, :, dsl],
                             start=(i == 0), stop=(i == len(steps) - 1),
                             perf_mode=DR)
        res = res_pool.tile([128, 512], BF, tag="res", name="res")
        nc.vector.tensor_copy(res[:], ps_f[:])
        nc.sync.dma_start(out_part[qc * 512 + j * 128:qc * 512 + (j + 1) * 128,
                                   dsl], res[:])

    def ph1_quanta(sc, xt):
        """Phase-1 chunk sc as a generator of ~8-matmul quanta. Chains are
        emitted in interleaved PAIRS at term-group granularity (2 open PSUM
        chains = pa bufs), so a chain stalled on a late-arriving DMA tile
        (w_lo / x_lo) never blocks the partner chain's ready work — matters
        for the DMA-paced first chunk."""
        ssl = slice(sc * 512, (sc + 1) * 512)

        def qk_chain(wn, h):
            hsl = slice(h * 128, (h + 1) * 128)
            ps = pa.tile([128, 512], F32, tag="pa", name="pqk")
            step = 0
            for part_w, part_x in (("h", "h"), ("l", "h"), ("h", "l")):
                for dp in range(NDP):
                    nc.tensor.matmul(ps[:], wt[wn, part_w][dp][:, :, hsl],
                                     xt[part_x, dp][:],
                                     start=(step == 0),
                                     stop=(step == 3 * NDP - 1),
                                     perf_mode=DR)
                    step += 1
                yield
            dstT = QT[h] if wn == "q" else KT[h]
            nc.scalar.mul(dstT[:, ssl], ps[:], DESCALE)

        def v_chain(j):
            kt = sc * 4 + j
            jsl = slice(j * 128, (j + 1) * 128)
            ps_v = pa.tile([128, E], F32, tag="pa", name="pv")
            step = 0
            for part_x, part_w in (("h", "h"), ("h", "l"), ("l", "h")):
                for dp in range(NDP):
                    nc.tensor.matmul(ps_v[:], xt[part_x, dp][:, :, jsl],
                                     wt["v", part_w][dp][:],
                                     start=(step == 0),
                                     stop=(step == 3 * NDP - 1),
                                     perf_mode=DR)
                    step += 1
                yield
            nc.scalar.mul(V[kt][:], ps_v[:], DESCALE)

        chains = [qk_chain("q", h) for h in range(HL)]
        chains += [qk_chain("k", h) for h in range(HL)]
        chains += [v_chain(j) for j in range(4)]
        for a, b in zip(chains[0::2], chains[1::2]):
            for ga, gb in zip(a, b):
                yield
                yield
            for _ in a:
                yield
            for _ in b:
                yield

    NQUANTA = HL * 2 * 3 + 4 * 3  # 36 quanta per phase-1 chunk

    # ---- main interleaved loop ------------------------------------------
    # iteration i: attention for qc=i-1, with phase-1 chunk sc=i and the
    # output projection of qc=i-2 spread through it as PE stall filler.
    for it in range(nsc + 1):
        sc = it if it < nsc else None
        qc = it - 1
        gen = None
        if sc is not None:
            xt = x_cur if sc == 0 else load_x_chunk(sc)
            gen = ph1_quanta(sc, xt)
        if qc < 0:
            for _ in gen:
                pass
            continue

        nkb = 4 * (qc + 1)
        nblocks = HL * nkb
        ot = {}
        for t in range(2):
            for part in ("h", "l"):
                ot[part, t] = o_pool.tile([128, 2, 512], F8, tag=f"o{part}{t}",
                                          name=f"o{part}{t}")
        o_tiles[qc] = ot
        blk_ctr = 0
        q_emitted = 0
        n_units = len(pending_units)
        u_emitted = 0
        for h in range(HL):
            hsl = slice(h * 128, (h + 1) * 128)
            ps_o = op.tile([128, 512], F32, tag="po", name="po")
            pden = dnp.tile([128, 4], F32, tag="pden", name="pden")
            # PSUM zeroing is 2KB-bank granular, so the four interleaved
            # per-column accumulation groups must not use start=True: memset
            # the bank once and accumulate with start=False.
            nc.vector.memset(pden[:], 0.0)
            for kb in range(nkb):
                kbloc = kb - 4 * qc
                s0 = max(0, kbloc * 128)
                ps_s = sp.tile([128, 512], F32, tag="ps", name="ps")
                nc.tensor.matmul(
                    ps_s[:, s0:], KT[h][:, kb * 128:(kb + 1) * 128],
                    QT[h][:, qc * 512 + s0:(qc + 1) * 512],
                    start=True, stop=True)
                es = es_pool.tile([128, 512], BF, tag="es", name="es")
                nc.scalar.activation(es[:, s0:], ps_s[:, s0:],
                                     mybir.ActivationFunctionType.Exp,
                                     bias=bias_t[:], scale=float(SCALE))
                if kbloc >= 0:
                    nc.vector.tensor_mul(es[:, s0:s0 + 128], es[:, s0:s0 + 128],
                                         mask_t[:])
                nc.tensor.matmul(ps_o[:, s0:], V[kb][:, hsl], es[:, s0:],
                                 start=(kb == 0), stop=(kb == nkb - 1))
                for j in range(max(0, kbloc), 4):
                    nc.tensor.matmul(pden[:, j:j + 1],
                                     es[:, j * 128:(j + 1) * 128], ones_t[:],
                                     start=False,
                                     stop=(kb == 4 * qc + j),
                                     skip_group_check=True)
                # proportional pacing of phase-1 quanta and outproj units
                blk_ctr += 1
                if gen is not None:
                    while q_emitted * nblocks < NQUANTA * blk_ctr:
                        if next(gen, None) is None:
                            break
                        q_emitted += 1
                while u_emitted * nblocks < n_units * blk_ctr:
                    emit_unit()
                    u_emitted += 1
            # normalization: den [128q,4] -> [1,512] strip -> recip -> bc
            # (f32r: same bits as f32, 1.5 instead of 2.0 transpose cycles/row)
            den_sb = nrm_pool.tile([128, 4], F32R, tag="den", name="den")
            nc.vector.tensor_copy(den_sb[:], pden[:])
            pt = ptp.tile([1, 512], F32R, tag="pt", name="pt")
            nc.vector.memset(pt[:].bitcast(F32), 0.0)
            for j in range(4):
                nc.tensor.matmul(pt[0:1, j * 128:(j + 1) * 128],
                                 den_sb[:, j:j + 1], eye_t[:],
                                 start=False, stop=True, is_transpose=True,
                                 skip_group_check=True)
            recip = nrm_pool.tile([1, 512], F32, tag="recip", name="recip")
            nc.vector.reciprocal(recip[:], pt[:])
            bc = nrm_pool.tile([128, 512], F32, tag="bc", name="bc")
            nc.gpsimd.partition_broadcast(bc[:], recip[0:1, :])
            of = nrm_pool.tile([128, 512], F32, tag="of", name="of")
            nc.vector.tensor_mul(of[:], ps_o[:], bc[:])
            t, i = h // 2, h % 2
            nc.vector.tensor_copy(ot["h", t][:, i, :], of[:])
            nc.vector.tensor_sub(ot["l", t][:, i, :], of[:], ot["h", t][:, i, :])
        if gen is not None:
            for _ in gen:
                pass
        while pending_units:
            emit_unit()
        pending_units += [(qc, j, dc) for j in range(4) for dc in range(4)]
        if qc == nsc - 1:
            while pending_units:
                emit_unit()
    ctx.close()


def shard_inputs(x, w_in, w_out, s=S):
    """Return the 8 per-core input dicts (host-side fp8 hi/lo packing)."""
    x = np.asarray(x, dtype=np.float32)
    w = np.asarray(w_in, dtype=np.float32).reshape(H, 3, DH, D)
    w_out = np.asarray(w_out, dtype=np.float32)

    def hilo(v):
        hi = v.astype(E4NP)
        lo = (v - hi.astype(np.float32)).astype(E4NP)
        return hi, lo

    def pack_d(v8, inner):
        # [D, inner] -> [D/256, 128, 2, inner]
        return np.ascontiguousarray(
            v8.reshape(NDP, 2, 128, inner).transpose(0, 2, 1, 3))

    eye = np.eye(128, dtype=np.float32)
    mask = np.triu(np.ones((128, 128), dtype=np.float32)).astype(BFNP)
    ones16 = np.full((128, 1), 1.0 / 16.0, dtype=np.float32).astype(BFNP)

    in_maps = []
    for core in range(8):
        b, g = divmod(core, 4)
        hs = slice(4 * g, 4 * g + HL)
        xT = np.ascontiguousarray(x[b, :s].T) * 16.0
        xh, xl = hilo(xT)
        m = {"xh8": pack_d(xh, s), "xl8": pack_d(xl, s),
             "eye": eye, "mask": mask, "ones16": ones16}
        for wi, wn in enumerate(("q", "k", "v")):
            wT = w[hs, wi].transpose(2, 0, 1).reshape(D, E) * 256.0
            wh, wl = hilo(wT)
            m[f"w{wn}h8"] = pack_d(wh, E)
            m[f"w{wn}l8"] = pack_d(wl, E)
        woT = w_out[:, 4 * g * DH:(4 * g + HL) * DH].T * 256.0  # [E, D]
        woh, wol = hilo(woT)
        m["woh8"] = np.ascontiguousarray(
            woh.reshape(2, 2, 128, D).transpose(0, 2, 1, 3))
        m["wol8"] = np.ascontiguousarray(
            wol.reshape(2, 2, 128, D).transpose(0, 2, 1, 3))
        in_maps.append(m)
    return in_maps


_prog_cache = {}


def get_program(s=S):
    if s not in _prog_cache:
        _prog_cache[s] = build_program(s)
    return _prog_cache[s]


def kernel(x, w_in, w_out):
    nc = get_program(S)
    in_maps = shard_inputs(x, w_in, w_out)
    res = run_bass_kernel_spmd(nc, in_maps, core_ids=list(range(8)))
    out = np.empty((B, S, D), dtype=np.float32)
    for b in range(B):
        acc = np.zeros((S, D), dtype=np.float64)
        for g in range(4):
            acc += res.results[4 * b + g]["out_part"]
        out[b] = (acc * DESCALE).astype(np.float32)
    return out


if __name__ == "__main__":
    import reference

    inputs = reference.setup_inputs()
    out = kernel(**{k: np.asarray(v) for k, v in inputs.items()})
    print("kernel output:", out.shape, out.dtype)


# revision 32
# speedup vs baseline: 1.0325x; 1.0325x over previous
"""Causal multi-head attention (B=2, S=2048, D=2048, H=16, DH=128) on 8 TRN2
NeuronCores.

Sharding: data-parallel over batch (2) x tensor-parallel over heads (4 groups
of 4 heads). Core c handles batch c//4, heads 4*(c%4) .. 4*(c%4)+3. Each core
computes its heads' attention and a partial output projection; the host sums
the 4 partials per batch (the "all-reduce") and applies the 2^-12 descale.

Numerics / speed scheme (validated to rel err ~1.7e-3 in emulation):
  - QKV projection: 3-term fp8e4 hi/lo DoubleRow matmuls (K=256 per instr,
    0.5 cycles/row -> 0.75x the f32r cycle count). Host pre-quantizes
    16*x and 256*w into packed pair layouts [D/256, 128, 2, *]; the 2^-12
    descale is applied at the PSUM->SBUF copy. Q,K,V stored bf16.
  - scores: bf16 matmuls (1 cycle/row, no N>=256 floor so diagonal blocks
    trim at 128 granularity), f32 PSUM.
  - softmax: exp on ACT with scale 1/sqrt(dh), bias -10 (scores bounded),
    es in bf16. Causal masking: 128-wide triangular mask multiply on the
    first valid 128 columns of diagonal blocks only.
  - denominators: es-as-stationary matmuls (out [128q, 1] per 128-q chunk,
    ~free on the PE), transposed back to a [1, 512] strip via 4 single-row
    PE transposes, reciprocal on DVE, gpsimd partition broadcast. The ones
    vector is 1/16 so bc = 16/den and O is produced at 16x scale for fp8.
  - PV: bf16 matmuls accumulating in PSUM.
  - output projection: 3-term fp8e4 hi/lo DoubleRow over E-pairs; O
    quantized to fp8 hi/lo pair tiles at 16x scale on DVE; host divides
    the summed partials by 4096 (= 16*256).
Phases are interleaved: attention for q-chunk qc runs right after the
projection chunk sc=qc (it needs exactly K/V chunks 0..qc), and the output
projection for qc-1 is spread through the attention of qc to fill PE stalls.
"""

import sys

if "/opt/trn_rl_repo" not in sys.path:
    sys.path.insert(0, "/opt/trn_rl_repo")

import numpy as np
import ml_dtypes

import concourse.bass as bass  # noqa: F401  (registers AP types)
import concourse.tile as tile
from concourse import bacc, mybir
from concourse.bass_utils import run_bass_kernel_spmd

B, S, D = 2, 2048, 2048
H, DH = 16, 128
HL = H // 4          # heads per core
E = HL * DH          # local feature width (512)
SCALE = 1.0 / np.sqrt(DH)
CBIAS = 10.0         # > max causal score (8.70 measured on the real inputs)

F32 = mybir.dt.float32
F32R = mybir.dt.float32r
F8 = mybir.dt.float8e4
BF = mybir.dt.bfloat16
DR = mybir.MatmulPerfMode.DoubleRow
E4NP = ml_dtypes.float8_e4m3
BFNP = ml_dtypes.bfloat16

NSC = S // 512       # s-chunks of 512
NDP = D // 256       # D-pair groups (8)
DESCALE = 2.0 ** -12  # 1/(16*256)
WARMUP = 16          # PE warm-up matmuls bridging the initial DMA window


def build_program(s=S):
    nsc = s // 512
    nc = bacc.Bacc("TRN2", target_bir_lowering=False, debug=False, num_devices=8)

    # dram layouts mirror the packed SBUF tiles exactly (contiguous per
    # partition), so each tensor loads with a single 2dim-collapsible DMA
    xh8 = nc.dram_tensor("xh8", [s // 512, 128, NDP, 2, 512], F8,
                         kind="ExternalInput").ap()
    xl8 = nc.dram_tensor("xl8", [s // 512, 128, NDP, 2, 512], F8,
                         kind="ExternalInput").ap()
    w8 = {}
    for wn in ("q", "k", "v"):
        for part in ("h", "l"):
            name = f"w{wn}{part}8"
            w8[wn, part] = nc.dram_tensor(name, [128, NDP, 2, E], F8,
                                          kind="ExternalInput").ap()
    woh8 = nc.dram_tensor("woh8", [128, 2, 2, D], F8, kind="ExternalInput").ap()
    wol8 = nc.dram_tensor("wol8", [128, 2, 2, D], F8, kind="ExternalInput").ap()
    eye = nc.dram_tensor("eye", [128, 128], F32R, kind="ExternalInput").ap()
    mask = nc.dram_tensor("mask", [128, 128], BF, kind="ExternalInput").ap()
    ones16 = nc.dram_tensor("ones16", [128, 1], BF, kind="ExternalInput").ap()
    out_part = nc.dram_tensor("out_part", [s, D], BF, kind="ExternalOutput").ap()

    with tile.TileContext(nc) as tc:
        _emit(tc, nc, xh8, xl8, w8, woh8, wol8, eye, mask, ones16, out_part, nsc)
    nc.compile()
    return nc


def _emit(tc, nc, xh8, xl8, w8, woh8, wol8, eye, mask, ones16, out_part, nsc):
    from contextlib import ExitStack
    ctx = ExitStack()
    s = nsc * 512

    # ---- constants / long-lived tiles -----------------------------------
    const_pool = ctx.enter_context(tc.tile_pool(name="const", bufs=1))
    bias_t = const_pool.tile([128, 1], F32, tag="bias", name="bias")
    nc.vector.memset(bias_t[:], -CBIAS)
    eye_t = const_pool.tile([128, 128], F32R, tag="eye", name="eye")
    mask_t = const_pool.tile([128, 128], BF, tag="mask", name="mask")
    ones_t = const_pool.tile([128, 1], BF, tag="ones", name="ones")

    # ---- PE warm-up during the initial DMA ramp -------------------------
    with tc.tile_pool(name="warm", bufs=1) as warm_pool, \
         tc.tile_pool(name="warmps", bufs=1, space="PSUM") as warm_ps:
        wsrc = warm_pool.tile([128, 512], F32, tag="wsrc", name="wsrc")
        nc.vector.memset(wsrc[:], 0.0)
        wps = warm_ps.tile([128, 512], F32, tag="wps", name="wps")
        for _ in range(WARMUP):
            nc.tensor.matmul(wps[:, :256], wsrc[:, :128], wsrc[:, :256],
                             start=True, stop=True)

    # ---- persistent products --------------------------------------------
    qk_pool = ctx.enter_context(tc.tile_pool(name="qk", bufs=1))
    QT = [qk_pool.tile([128, s], BF, tag=f"qT{h}", name=f"qT{h}") for h in range(HL)]
    KT = [qk_pool.tile([128, s], BF, tag=f"kT{h}", name=f"kT{h}") for h in range(HL)]
    v_pool = ctx.enter_context(tc.tile_pool(name="v", bufs=1))
    V = [v_pool.tile([128, E], BF, tag=f"v{kt}", name=f"v{kt}")
         for kt in range(4 * nsc)]

    # ---- weights (one batched tile + one DMA per tensor: the HWDGE
    # descriptor-generation slot costs ~625ns per dma_start, so many small
    # tile DMAs serialize the startup) ------------------------------------
    w_pool = ctx.enter_context(tc.tile_pool(name="w", bufs=1))
    wt = {}
    for wn in ("q", "k", "v"):
        for part in ("h", "l"):
            wt[wn, part] = w_pool.tile([128, NDP, 2, E], F8,
                                       tag=f"w{wn}{part}", name=f"w{wn}{part}")
    wo_pool = ctx.enter_context(tc.tile_pool(name="wo", bufs=1))
    wo_t = {}
    for part in ("h", "l"):
        wo_t[part] = wo_pool.tile([128, 2, 2, D], F8, tag=f"wo{part}",
                                  name=f"wo{part}")

    # O pair tiles (fp8 hi/lo, 16x scale), per head-pair t, double-buffered
    # across qc (outproj of qc-1 runs during attention of qc).
    o_pool = ctx.enter_context(tc.tile_pool(name="o", bufs=2))

    # ---- phase-2 pools ---------------------------------------------------
    es_pool = ctx.enter_context(tc.tile_pool(name="es", bufs=6))
    nrm_pool = ctx.enter_context(tc.tile_pool(name="nrm", bufs=2))
    res_pool = ctx.enter_context(tc.tile_pool(name="res", bufs=4))
    x_pool = ctx.enter_context(tc.tile_pool(name="x", bufs=2))

    pa = ctx.enter_context(tc.tile_pool(name="pa", bufs=2, space="PSUM"))
    sp = ctx.enter_context(tc.tile_pool(name="sp", bufs=2, space="PSUM"))
    op = ctx.enter_context(tc.tile_pool(name="op", bufs=2, space="PSUM"))
    dnp = ctx.enter_context(tc.tile_pool(name="dnp", bufs=1, space="PSUM"))
    ptp = ctx.enter_context(tc.tile_pool(name="ptp", bufs=1, space="PSUM"))

    # ---- initial DMAs (batched, priority ordered) ------------------------
    xdram = {"h": xh8, "l": xl8}

    def load_x_chunk(sc):
        xt = {}
        for part in ("h", "l"):
            t = x_pool.tile([128, NDP, 2, 512], F8, tag=f"x{part}",
                            name=f"x{part}")
            nc.sync.dma_start(t[:], xdram[part][sc])
            xt[part] = t
        return xt

    x_cur = {}
    nc.sync.dma_start(wt["q", "h"][:], w8["q", "h"])
    t = x_pool.tile([128, NDP, 2, 512], F8, tag="xh", name="xh")
    nc.sync.dma_start(t[:], xh8[0])
    x_cur["h"] = t
    nc.sync.dma_start(wt["q", "l"][:], w8["q", "l"])
    t = x_pool.tile([128, NDP, 2, 512], F8, tag="xl", name="xl")
    nc.sync.dma_start(t[:], xl8[0])
    x_cur["l"] = t
    nc.sync.dma_start(wt["k", "h"][:], w8["k", "h"])
    nc.sync.dma_start(wt["k", "l"][:], w8["k", "l"])
    nc.sync.dma_start(wt["v", "h"][:], w8["v", "h"])
    nc.sync.dma_start(wt["v", "l"][:], w8["v", "l"])
    nc.sync.dma_start(eye_t[:], eye)
    nc.sync.dma_start(mask_t[:], mask)
    nc.sync.dma_start(ones_t[:], ones16)
    nc.sync.dma_start(wo_t["h"][:], woh8)
    nc.sync.dma_start(wo_t["l"][:], wol8)

    # pending output-projection units: (qc, j, dc) tuples
    pending_units = []
    o_tiles = {}   # qc -> {("h"|"l", t): tile}

    res_cur = [None]

    def emit_unit():
        if not pending_units:
            return
        qc, j, dc = pending_units.pop(0)
        ot = o_tiles[qc]
        jsl = slice(j * 128, (j + 1) * 128)
        dsl = slice(dc * 512, (dc + 1) * 512)
        ps_f = pa.tile([128, 512], F32, tag="pa", name="pf")
        steps = []
        for t in range(2):
            steps += [(ot["h", t], wo_t["h"], t), (ot["h", t], wo_t["l"], t),
                      (ot["l", t], wo_t["h"], t)]
        for i, (o8, w8t, t) in enumerate(steps):
            nc.tensor.matmul(ps_f[:], o8[:, :, jsl], w8t[:, t, :, dsl],
                             start=(i == 0), stop=(i == len(steps) - 1),
                             perf_mode=DR)
        # batch the 4 d-chunks of one 128-row block into a single out DMA
        if dc == 0:
            res_cur[0] = res_pool.tile([128, D], BF, tag="res", name="res")
        nc.vector.tensor_copy(res_cur[0][:, dsl], ps_f[:])
        if dc == 3:
            nc.sync.dma_start(
                out_part[qc * 512 + j * 128:qc * 512 + (j + 1) * 128, :],
                res_cur[0][:])

    def ph1_quanta(sc, xt):
        """Phase-1 chunk sc as a generator of ~8-matmul quanta. Chains are
        emitted in interleaved PAIRS at term-group granularity (2 open PSUM
        chains = pa bufs), so a chain stalled on a late-arriving DMA tile
        (w_lo / x_lo) never blocks the partner chain's ready work — matters
        for the DMA-paced first chunk."""
        ssl = slice(sc * 512, (sc + 1) * 512)

        def qk_chain(wn, h):
            hsl = slice(h * 128, (h + 1) * 128)
            ps = pa.tile([128, 512], F32, tag="pa", name="pqk")
            step = 0
            for part_w, part_x in (("h", "h"), ("l", "h"), ("h", "l")):
                for dp in range(NDP):
                    nc.tensor.matmul(ps[:], wt[wn, part_w][:, dp, :, hsl],
                                     xt[part_x][:, dp],
                                     start=(step == 0),
                                     stop=(step == 3 * NDP - 1),
                                     perf_mode=DR)
                    step += 1
                yield
            dstT = QT[h] if wn == "q" else KT[h]
            nc.scalar.mul(dstT[:, ssl], ps[:], DESCALE)

        def v_chain(j):
            kt = sc * 4 + j
            jsl = slice(j * 128, (j + 1) * 128)
            ps_v = pa.tile([128, E], F32, tag="pa", name="pv")
            step = 0
            for part_x, part_w in (("h", "h"), ("h", "l"), ("l", "h")):
                for dp in range(NDP):
                    nc.tensor.matmul(ps_v[:], xt[part_x][:, dp, :, jsl],
                                     wt["v", part_w][:, dp],
                                     start=(step == 0),
                                     stop=(step == 3 * NDP - 1),
                                     perf_mode=DR)
                    step += 1
                yield
            nc.scalar.mul(V[kt][:], ps_v[:], DESCALE)

        chains = [qk_chain("q", h) for h in range(HL)]
        chains += [qk_chain("k", h) for h in range(HL)]
        chains += [v_chain(j) for j in range(4)]
        for a, b in zip(chains[0::2], chains[1::2]):
            for ga, gb in zip(a, b):
                yield
                yield
            for _ in a:
                yield
            for _ in b:
                yield

    NQUANTA = HL * 2 * 3 + 4 * 3  # 36 quanta per phase-1 chunk

    # ---- main interleaved loop ------------------------------------------
    # iteration i: attention for qc=i-1, with phase-1 chunk sc=i and the
    # output projection of qc=i-2 spread through it as PE stall filler.
    for it in range(nsc + 1):
        sc = it if it < nsc else None
        qc = it - 1
        gen = None
        if sc is not None:
            xt = x_cur if sc == 0 else load_x_chunk(sc)
            gen = ph1_quanta(sc, xt)
        if qc < 0:
            for _ in gen:
                pass
            continue

        nkb = 4 * (qc + 1)
        nblocks = HL * nkb
        ot = {}
        for t in range(2):
            for part in ("h", "l"):
                ot[part, t] = o_pool.tile([128, 2, 512], F8, tag=f"o{part}{t}",
                                          name=f"o{part}{t}")
        o_tiles[qc] = ot
        blk_ctr = 0
        q_emitted = 0
        n_units = len(pending_units)
        u_emitted = 0
        for h in range(HL):
            hsl = slice(h * 128, (h + 1) * 128)
            ps_o = op.tile([128, 512], F32, tag="po", name="po")
            pden = dnp.tile([128, 4], F32, tag="pden", name="pden")
            # PSUM zeroing is 2KB-bank granular, so the four interleaved
            # per-column accumulation groups must not use start=True: memset
            # the bank once and accumulate with start=False.
            nc.vector.memset(pden[:], 0.0)
            for kb in range(nkb):
                kbloc = kb - 4 * qc
                s0 = max(0, kbloc * 128)
                ps_s = sp.tile([128, 512], F32, tag="ps", name="ps")
                nc.tensor.matmul(
                    ps_s[:, s0:], KT[h][:, kb * 128:(kb + 1) * 128],
                    QT[h][:, qc * 512 + s0:(qc + 1) * 512],
                    start=True, stop=True)
                es = es_pool.tile([128, 512], BF, tag="es", name="es")
                nc.scalar.activation(es[:, s0:], ps_s[:, s0:],
                                     mybir.ActivationFunctionType.Exp,
                                     bias=bias_t[:], scale=float(SCALE))
                if kbloc >= 0:
                    nc.vector.tensor_mul(es[:, s0:s0 + 128], es[:, s0:s0 + 128],
                                         mask_t[:])
                nc.tensor.matmul(ps_o[:, s0:], V[kb][:, hsl], es[:, s0:],
                                 start=(kb == 0), stop=(kb == nkb - 1))
                for j in range(max(0, kbloc), 4):
                    nc.tensor.matmul(pden[:, j:j + 1],
                                     es[:, j * 128:(j + 1) * 128], ones_t[:],
                                     start=False,
                                     stop=(kb == 4 * qc + j),
                                     skip_group_check=True)
                # proportional pacing of phase-1 quanta and outproj units
                blk_ctr += 1
                if gen is not None:
                    while q_emitted * nblocks < NQUANTA * blk_ctr:
                        if next(gen, None) is None:
                            break
                        q_emitted += 1
                while u_emitted * nblocks < n_units * blk_ctr:
                    emit_unit()
                    u_emitted += 1
            # normalization: den [128q,4] -> [1,512] strip -> recip -> bc
            # (f32r: same bits as f32, 1.5 instead of 2.0 transpose cycles/row)
            den_sb = nrm_pool.tile([128, 4], F32R, tag="den", name="den")
            nc.vector.tensor_copy(den_sb[:], pden[:])
            pt = ptp.tile([1, 512], F32R, tag="pt", name="pt")
            nc.vector.memset(pt[:].bitcast(F32), 0.0)
            for j in range(4):
                nc.tensor.matmul(pt[0:1, j * 128:(j + 1) * 128],
                                 den_sb[:, j:j + 1], eye_t[:],
                                 start=False, stop=True, is_transpose=True,
                                 skip_group_check=True)
            recip = nrm_pool.tile([1, 512], F32, tag="recip", name="recip")
            nc.vector.reciprocal(recip[:], pt[:])
            bc = nrm_pool.tile([128, 512], F32, tag="bc", name="bc")
            nc.gpsimd.partition_broadcast(bc[:], recip[0:1, :])
            of = nrm_pool.tile([128, 512], F32, tag="of", name="of")
            nc.vector.tensor_mul(of[:], ps_o[:], bc[:])
            t, i = h // 2, h % 2
            nc.vector.tensor_copy(ot["h", t][:, i, :], of[:])
            nc.vector.tensor_sub(ot["l", t][:, i, :], of[:], ot["h", t][:, i, :])
        if gen is not None:
            for _ in gen:
                pass
        while pending_units:
            emit_unit()
        pending_units += [(qc, j, dc) for j in range(4) for dc in range(4)]
        if qc == nsc - 1:
            while pending_units:
                emit_unit()
    ctx.close()


def shard_inputs(x, w_in, w_out, s=S):
    """Return the 8 per-core input dicts (host-side fp8 hi/lo packing)."""
    x = np.asarray(x, dtype=np.float32)
    w = np.asarray(w_in, dtype=np.float32).reshape(H, 3, DH, D)
    w_out = np.asarray(w_out, dtype=np.float32)

    def hilo(v):
        hi = v.astype(E4NP)
        lo = (v - hi.astype(np.float32)).astype(E4NP)
        return hi, lo

    def pack_w(v8):
        # [D, E] -> [128(p), NDP, 2(i), E]  (contiguous per partition)
        return np.ascontiguousarray(
            v8.reshape(NDP, 2, 128, E).transpose(2, 0, 1, 3))

    def pack_x(v8, s):
        # [D, s] -> [s/512(sc), 128(p), NDP, 2(i), 512]
        return np.ascontiguousarray(
            v8.reshape(NDP, 2, 128, s // 512, 512).transpose(3, 2, 0, 1, 4))

    eye = np.eye(128, dtype=np.float32)
    mask = np.triu(np.ones((128, 128), dtype=np.float32)).astype(BFNP)
    ones16 = np.full((128, 1), 1.0 / 16.0, dtype=np.float32).astype(BFNP)

    in_maps = []
    for core in range(8):
        b, g = divmod(core, 4)
        hs = slice(4 * g, 4 * g + HL)
        xT = np.ascontiguousarray(x[b, :s].T) * 16.0
        xh, xl = hilo(xT)
        m = {"xh8": pack_x(xh, s), "xl8": pack_x(xl, s),
             "eye": eye, "mask": mask, "ones16": ones16}
        for wi, wn in enumerate(("q", "k", "v")):
            wT = w[hs, wi].transpose(2, 0, 1).reshape(D, E) * 256.0
            wh, wl = hilo(wT)
            m[f"w{wn}h8"] = pack_w(wh)
            m[f"w{wn}l8"] = pack_w(wl)
        woT = w_out[:, 4 * g * DH:(4 * g + HL) * DH].T * 256.0  # [E, D]
        woh, wol = hilo(woT)
        # [E, D] -> [128(p), 2(tp), 2(i), D]
        m["woh8"] = np.ascontiguousarray(
            woh.reshape(2, 2, 128, D).transpose(2, 0, 1, 3))
        m["wol8"] = np.ascontiguousarray(
            wol.reshape(2, 2, 128, D).transpose(2, 0, 1, 3))
        in_maps.append(m)
    return in_maps


_prog_cache = {}


def get_program(s=S):
    if s not in _prog_cache:
        _prog_cache[s] = build_program(s)
    return _prog_cache[s]


def kernel(x, w_in, w_out):
    nc = get_program(S)
    in_maps = shard_inputs(x, w_in, w_out)
    res = run_bass_kernel_spmd(nc, in_maps, core_ids=list(range(8)))
    out = np.empty((B, S, D), dtype=np.float32)
    for b in range(B):
        acc = np.zeros((S, D), dtype=np.float64)
        for g in range(4):
            acc += res.results[4 * b + g]["out_part"]
        out[b] = (acc * DESCALE).astype(np.float32)
    return out


if __name__ == "__main__":
    import reference

    inputs = reference.setup_inputs()
    out = kernel(**{k: np.asarray(v) for k, v in inputs.items()})
    print("kernel output:", out.shape, out.dtype)


# revision 33
# speedup vs baseline: 1.0467x; 1.0137x over previous
"""Causal multi-head attention (B=2, S=2048, D=2048, H=16, DH=128) on 8 TRN2
NeuronCores.

Sharding: data-parallel over batch (2) x tensor-parallel over heads (4 groups
of 4 heads). Core c handles batch c//4, heads 4*(c%4) .. 4*(c%4)+3. Each core
computes its heads' attention and a partial output projection; the host sums
the 4 partials per batch (the "all-reduce") and applies the 2^-12 descale.

Numerics / speed scheme (validated to rel err ~1.7e-3 in emulation):
  - QKV projection: 3-term fp8e4 hi/lo DoubleRow matmuls (K=256 per instr,
    0.5 cycles/row -> 0.75x the f32r cycle count). Host pre-quantizes
    16*x and 256*w into packed pair layouts [D/256, 128, 2, *]; the 2^-12
    descale is applied at the PSUM->SBUF copy. Q,K,V stored bf16.
  - scores: bf16 matmuls (1 cycle/row, no N>=256 floor so diagonal blocks
    trim at 128 granularity), f32 PSUM.
  - softmax: exp on ACT with scale 1/sqrt(dh), bias -10 (scores bounded),
    es in bf16. Causal masking: 128-wide triangular mask multiply on the
    first valid 128 columns of diagonal blocks only.
  - denominators: es-as-stationary matmuls (out [128q, 1] per 128-q chunk,
    ~free on the PE), transposed back to a [1, 512] strip via 4 single-row
    PE transposes, reciprocal on DVE, gpsimd partition broadcast. The ones
    vector is 1/16 so bc = 16/den and O is produced at 16x scale for fp8.
  - PV: bf16 matmuls accumulating in PSUM.
  - output projection: 3-term fp8e4 hi/lo DoubleRow over E-pairs; O
    quantized to fp8 hi/lo pair tiles at 16x scale on DVE; host divides
    the summed partials by 4096 (= 16*256).
Phases are interleaved: attention for q-chunk qc runs right after the
projection chunk sc=qc (it needs exactly K/V chunks 0..qc), and the output
projection for qc-1 is spread through the attention of qc to fill PE stalls.
"""

import sys

if "/opt/trn_rl_repo" not in sys.path:
    sys.path.insert(0, "/opt/trn_rl_repo")

import numpy as np
import ml_dtypes

import concourse.bass as bass  # noqa: F401  (registers AP types)
import concourse.tile as tile
from concourse import bacc, mybir
from concourse.bass_utils import run_bass_kernel_spmd

B, S, D = 2, 2048, 2048
H, DH = 16, 128
HL = H // 4          # heads per core
E = HL * DH          # local feature width (512)
SCALE = 1.0 / np.sqrt(DH)
CBIAS = 10.0         # > max causal score (8.70 measured on the real inputs)

F32 = mybir.dt.float32
F32R = mybir.dt.float32r
F8 = mybir.dt.float8e4
BF = mybir.dt.bfloat16
DR = mybir.MatmulPerfMode.DoubleRow
E4NP = ml_dtypes.float8_e4m3
BFNP = ml_dtypes.bfloat16

NSC = S // 512       # s-chunks of 512
NDP = D // 256       # D-pair groups (8)
DESCALE = 2.0 ** -12  # 1/(16*256)
WARMUP = 16          # PE warm-up matmuls bridging the initial DMA window


def build_program(s=S):
    nsc = s // 512
    nc = bacc.Bacc("TRN2", target_bir_lowering=False, debug=False, num_devices=8)

    # dram layouts mirror the packed SBUF tiles exactly (contiguous per
    # partition), so each tensor loads with a single 2dim-collapsible DMA
    xh8 = nc.dram_tensor("xh8", [s // 512, 128, NDP, 2, 512], F8,
                         kind="ExternalInput").ap()
    xl8 = nc.dram_tensor("xl8", [s // 512, 128, NDP, 2, 512], F8,
                         kind="ExternalInput").ap()
    w8 = {}
    for wn in ("q", "k", "v"):
        for part in ("h", "l"):
            name = f"w{wn}{part}8"
            w8[wn, part] = nc.dram_tensor(name, [128, NDP, 2, E], F8,
                                          kind="ExternalInput").ap()
    woh8 = nc.dram_tensor("woh8", [128, 2, 2, D], F8, kind="ExternalInput").ap()
    wol8 = nc.dram_tensor("wol8", [128, 2, 2, D], F8, kind="ExternalInput").ap()
    eye = nc.dram_tensor("eye", [128, 128], F32R, kind="ExternalInput").ap()
    mask = nc.dram_tensor("mask", [128, 128], BF, kind="ExternalInput").ap()
    ones16 = nc.dram_tensor("ones16", [128, 1], BF, kind="ExternalInput").ap()
    out_part = nc.dram_tensor("out_part", [s, D], BF, kind="ExternalOutput").ap()

    with tile.TileContext(nc) as tc:
        _emit(tc, nc, xh8, xl8, w8, woh8, wol8, eye, mask, ones16, out_part, nsc)
    nc.compile()
    return nc


def _emit(tc, nc, xh8, xl8, w8, woh8, wol8, eye, mask, ones16, out_part, nsc):
    from contextlib import ExitStack
    ctx = ExitStack()
    s = nsc * 512

    # ---- constants / long-lived tiles -----------------------------------
    const_pool = ctx.enter_context(tc.tile_pool(name="const", bufs=1))
    bias_t = const_pool.tile([128, 1], F32, tag="bias", name="bias")
    nc.vector.memset(bias_t[:], -CBIAS)
    eye_t = const_pool.tile([128, 128], F32R, tag="eye", name="eye")
    mask_t = const_pool.tile([128, 128], BF, tag="mask", name="mask")
    ones_t = const_pool.tile([128, 1], BF, tag="ones", name="ones")

    # ---- PE warm-up during the initial DMA ramp -------------------------
    with tc.tile_pool(name="warm", bufs=1) as warm_pool, \
         tc.tile_pool(name="warmps", bufs=1, space="PSUM") as warm_ps:
        wsrc = warm_pool.tile([128, 512], F32, tag="wsrc", name="wsrc")
        nc.vector.memset(wsrc[:], 0.0)
        wps = warm_ps.tile([128, 512], F32, tag="wps", name="wps")
        for _ in range(WARMUP):
            nc.tensor.matmul(wps[:, :256], wsrc[:, :128], wsrc[:, :256],
                             start=True, stop=True)

    # ---- persistent products --------------------------------------------
    qk_pool = ctx.enter_context(tc.tile_pool(name="qk", bufs=1))
    QT = [qk_pool.tile([128, s], BF, tag=f"qT{h}", name=f"qT{h}") for h in range(HL)]
    KT = [qk_pool.tile([128, s], BF, tag=f"kT{h}", name=f"kT{h}") for h in range(HL)]
    v_pool = ctx.enter_context(tc.tile_pool(name="v", bufs=1))
    V = [v_pool.tile([128, E], BF, tag=f"v{kt}", name=f"v{kt}")
         for kt in range(4 * nsc)]

    # ---- weights (one batched tile + one DMA per tensor: the HWDGE
    # descriptor-generation slot costs ~625ns per dma_start, so many small
    # tile DMAs serialize the startup) ------------------------------------
    w_pool = ctx.enter_context(tc.tile_pool(name="w", bufs=1))
    wt = {}
    for wn in ("q", "k", "v"):
        for part in ("h", "l"):
            wt[wn, part] = w_pool.tile([128, NDP, 2, E], F8,
                                       tag=f"w{wn}{part}", name=f"w{wn}{part}")
    wo_pool = ctx.enter_context(tc.tile_pool(name="wo", bufs=1))
    wo_t = {}
    for part in ("h", "l"):
        wo_t[part] = wo_pool.tile([128, 2, 2, D], F8, tag=f"wo{part}",
                                  name=f"wo{part}")

    # O pair tiles (fp8 hi/lo, 16x scale), per head-pair t, double-buffered
    # across qc (outproj of qc-1 runs during attention of qc).
    o_pool = ctx.enter_context(tc.tile_pool(name="o", bufs=2))

    # ---- phase-2 pools ---------------------------------------------------
    es_pool = ctx.enter_context(tc.tile_pool(name="es", bufs=6))
    nrm_pool = ctx.enter_context(tc.tile_pool(name="nrm", bufs=2))
    res_pool = ctx.enter_context(tc.tile_pool(name="res", bufs=4))
    x_pool = ctx.enter_context(tc.tile_pool(name="x", bufs=2))

    pa = ctx.enter_context(tc.tile_pool(name="pa", bufs=2, space="PSUM"))
    sp = ctx.enter_context(tc.tile_pool(name="sp", bufs=2, space="PSUM"))
    op = ctx.enter_context(tc.tile_pool(name="op", bufs=2, space="PSUM"))
    dnp = ctx.enter_context(tc.tile_pool(name="dnp", bufs=1, space="PSUM"))
    ptp = ctx.enter_context(tc.tile_pool(name="ptp", bufs=1, space="PSUM"))

    # ---- initial DMAs (batched, priority ordered) ------------------------
    xdram = {"h": xh8, "l": xl8}

    def load_x_chunk(sc):
        xt = {}
        for part in ("h", "l"):
            t = x_pool.tile([128, NDP, 2, 512], F8, tag=f"x{part}",
                            name=f"x{part}")
            nc.sync.dma_start(t[:], xdram[part][sc])
            xt[part] = t
        return xt

    x_cur = {}
    nc.sync.dma_start(wt["q", "h"][:], w8["q", "h"])
    t = x_pool.tile([128, NDP, 2, 512], F8, tag="xh", name="xh")
    nc.sync.dma_start(t[:], xh8[0])
    x_cur["h"] = t
    nc.sync.dma_start(wt["q", "l"][:], w8["q", "l"])
    t = x_pool.tile([128, NDP, 2, 512], F8, tag="xl", name="xl")
    nc.sync.dma_start(t[:], xl8[0])
    x_cur["l"] = t
    nc.sync.dma_start(wt["k", "h"][:], w8["k", "h"])
    nc.sync.dma_start(wt["k", "l"][:], w8["k", "l"])
    nc.sync.dma_start(wt["v", "h"][:], w8["v", "h"])
    nc.sync.dma_start(wt["v", "l"][:], w8["v", "l"])
    nc.sync.dma_start(eye_t[:], eye)
    nc.sync.dma_start(mask_t[:], mask)
    nc.sync.dma_start(ones_t[:], ones16)
    nc.sync.dma_start(wo_t["h"][:], woh8)
    nc.sync.dma_start(wo_t["l"][:], wol8)

    # pending output-projection units: (qc, j, dc) tuples
    pending_units = []
    o_tiles = {}   # qc -> {("h"|"l", t): tile}

    res_cur = [None]

    def emit_unit():
        if not pending_units:
            return
        qc, j, dc = pending_units.pop(0)
        ot = o_tiles[qc]
        jsl = slice(j * 128, (j + 1) * 128)
        dsl = slice(dc * 512, (dc + 1) * 512)
        ps_f = pa.tile([128, 512], F32, tag="pa", name="pf")
        steps = []
        for t in range(2):
            steps += [(ot["h", t], wo_t["h"], t), (ot["h", t], wo_t["l"], t),
                      (ot["l", t], wo_t["h"], t)]
        for i, (o8, w8t, t) in enumerate(steps):
            nc.tensor.matmul(ps_f[:], o8[:, :, jsl], w8t[:, t, :, dsl],
                             start=(i == 0), stop=(i == len(steps) - 1),
                             perf_mode=DR)
        # batch the 4 d-chunks of one 128-row block into a single out DMA
        if dc == 0:
            res_cur[0] = res_pool.tile([128, D], BF, tag="res", name="res")
        nc.vector.tensor_copy(res_cur[0][:, dsl], ps_f[:])
        if dc == 3:
            nc.sync.dma_start(
                out_part[qc * 512 + j * 128:qc * 512 + (j + 1) * 128, :],
                res_cur[0][:])

    def ph1_quanta(sc, xt):
        """Phase-1 chunk sc as a generator of ~8-matmul quanta. Chains are
        emitted in interleaved PAIRS at term-group granularity (2 open PSUM
        chains = pa bufs), so a chain stalled on a late-arriving DMA tile
        (w_lo / x_lo) never blocks the partner chain's ready work — matters
        for the DMA-paced first chunk."""
        ssl = slice(sc * 512, (sc + 1) * 512)

        def qk_chain(wn, h):
            hsl = slice(h * 128, (h + 1) * 128)
            ps = pa.tile([128, 512], F32, tag="pa", name="pqk")
            step = 0
            for part_w, part_x in (("h", "h"), ("l", "h"), ("h", "l")):
                for dp in range(NDP):
                    nc.tensor.matmul(ps[:], wt[wn, part_w][:, dp, :, hsl],
                                     xt[part_x][:, dp],
                                     start=(step == 0),
                                     stop=(step == 3 * NDP - 1),
                                     perf_mode=DR)
                    step += 1
                yield
            dstT = QT[h] if wn == "q" else KT[h]
            nc.scalar.mul(dstT[:, ssl], ps[:], DESCALE)

        def v_chain(j):
            kt = sc * 4 + j
            jsl = slice(j * 128, (j + 1) * 128)
            ps_v = pa.tile([128, E], F32, tag="pa", name="pv")
            step = 0
            for part_x, part_w in (("h", "h"), ("h", "l"), ("l", "h")):
                for dp in range(NDP):
                    nc.tensor.matmul(ps_v[:], xt[part_x][:, dp, :, jsl],
                                     wt["v", part_w][:, dp],
                                     start=(step == 0),
                                     stop=(step == 3 * NDP - 1),
                                     perf_mode=DR)
                    step += 1
                yield
            nc.scalar.mul(V[kt][:], ps_v[:], DESCALE)

        chains = [qk_chain("q", h) for h in range(HL)]
        chains += [qk_chain("k", h) for h in range(HL)]
        chains += [v_chain(j) for j in range(4)]
        for a, b in zip(chains[0::2], chains[1::2]):
            for ga, gb in zip(a, b):
                yield
                yield
            for _ in a:
                yield
            for _ in b:
                yield

    NQUANTA = HL * 2 * 3 + 4 * 3  # 36 quanta per phase-1 chunk
    EARLY = 1                     # heads of att(qc) pulled into iteration qc

    def get_o_tiles(qc):
        if qc not in o_tiles:
            ot = {}
            for t in range(2):
                for part in ("h", "l"):
                    ot[part, t] = o_pool.tile([128, 2, 512], F8,
                                              tag=f"o{part}{t}",
                                              name=f"o{part}{t}")
            o_tiles[qc] = ot
        return o_tiles[qc]

    def att_head(qc, h, block_cb):
        """Attention for (head h, q-chunk qc); block_cb() paces filler work
        (phase-1 quanta / outproj units) after each k-block."""
        ot = get_o_tiles(qc)
        nkb = 4 * (qc + 1)
        hsl = slice(h * 128, (h + 1) * 128)
        ps_o = op.tile([128, 512], F32, tag="po", name="po")
        pden = dnp.tile([128, 4], F32, tag="pden", name="pden")
        # PSUM zeroing is 2KB-bank granular, so the four interleaved
        # per-column accumulation groups must not use start=True: memset
        # the bank once and accumulate with start=False.
        nc.vector.memset(pden[:], 0.0)
        for kb in range(nkb):
            kbloc = kb - 4 * qc
            s0 = max(0, kbloc * 128)
            ps_s = sp.tile([128, 512], F32, tag="ps", name="ps")
            nc.tensor.matmul(
                ps_s[:, s0:], KT[h][:, kb * 128:(kb + 1) * 128],
                QT[h][:, qc * 512 + s0:(qc + 1) * 512],
                start=True, stop=True)
            es = es_pool.tile([128, 512], BF, tag="es", name="es")
            nc.scalar.activation(es[:, s0:], ps_s[:, s0:],
                                 mybir.ActivationFunctionType.Exp,
                                 bias=bias_t[:], scale=float(SCALE))
            if kbloc >= 0:
                nc.vector.tensor_mul(es[:, s0:s0 + 128], es[:, s0:s0 + 128],
                                     mask_t[:])
            nc.tensor.matmul(ps_o[:, s0:], V[kb][:, hsl], es[:, s0:],
                             start=(kb == 0), stop=(kb == nkb - 1))
            for j in range(max(0, kbloc), 4):
                nc.tensor.matmul(pden[:, j:j + 1],
                                 es[:, j * 128:(j + 1) * 128], ones_t[:],
                                 start=False,
                                 stop=(kb == 4 * qc + j),
                                 skip_group_check=True)
            block_cb()
        # normalization: den [128q,4] -> [1,512] strip -> recip -> bc
        # (f32r: same bits as f32, 1.5 instead of 2.0 transpose cycles/row)
        den_sb = nrm_pool.tile([128, 4], F32R, tag="den", name="den")
        nc.vector.tensor_copy(den_sb[:], pden[:])
        pt = ptp.tile([1, 512], F32R, tag="pt", name="pt")
        nc.vector.memset(pt[:].bitcast(F32), 0.0)
        for j in range(4):
            nc.tensor.matmul(pt[0:1, j * 128:(j + 1) * 128],
                             den_sb[:, j:j + 1], eye_t[:],
                             start=False, stop=True, is_transpose=True,
                             skip_group_check=True)
        recip = nrm_pool.tile([1, 512], F32, tag="recip", name="recip")
        nc.vector.reciprocal(recip[:], pt[:])
        bc = nrm_pool.tile([128, 512], F32, tag="bc", name="bc")
        nc.gpsimd.partition_broadcast(bc[:], recip[0:1, :])
        of = nrm_pool.tile([128, 512], F32, tag="of", name="of")
        nc.vector.tensor_mul(of[:], ps_o[:], bc[:])
        t, i = h // 2, h % 2
        nc.vector.tensor_copy(ot["h", t][:, i, :], of[:])
        nc.vector.tensor_sub(ot["l", t][:, i, :], of[:], ot["h", t][:, i, :])

    # ---- main interleaved loop ------------------------------------------
    # iteration it: heads EARLY..4 of att(qc=it-1), phase-1 chunk sc=it
    # spread through them, then heads 0..EARLY of att(qc=it) right after the
    # phase-1 flush (pulling exp/ACT load out of the tail iteration).
    for it in range(nsc + 1):
        sc = it if it < nsc else None
        qc = it - 1
        gen = None
        if sc is not None:
            xt = x_cur if sc == 0 else load_x_chunk(sc)
            gen = ph1_quanta(sc, xt)

        main_heads = [] if qc < 0 else [(qc, h) for h in range(
            0 if qc == 0 else EARLY, HL)]
        early_heads = [] if sc is None else [(sc, h) for h in range(EARLY)]
        nblocks = (sum(4 * (q + 1) for q, _ in main_heads)
                   + sum(4 * (q + 1) for q, _ in early_heads))
        state = {"blk": 0, "q": 0, "u": 0}
        n_units = len(pending_units)

        def block_cb():
            state["blk"] += 1
            if gen is not None:
                while state["q"] * nblocks < NQUANTA * state["blk"]:
                    if next(gen, None) is None:
                        break
                    state["q"] += 1
            while state["u"] * nblocks < n_units * state["blk"]:
                emit_unit()
                state["u"] += 1

        for q, h in main_heads:
            att_head(q, h, block_cb)
        if gen is not None:
            for _ in gen:
                pass
        for q, h in early_heads:
            att_head(q, h, block_cb)
        while pending_units:
            emit_unit()
        if qc >= 0:
            pending_units += [(qc, j, dc) for j in range(4) for dc in range(4)]
        if qc == nsc - 1:
            while pending_units:
                emit_unit()
    ctx.close()


def shard_inputs(x, w_in, w_out, s=S):
    """Return the 8 per-core input dicts (host-side fp8 hi/lo packing)."""
    x = np.asarray(x, dtype=np.float32)
    w = np.asarray(w_in, dtype=np.float32).reshape(H, 3, DH, D)
    w_out = np.asarray(w_out, dtype=np.float32)

    def hilo(v):
        hi = v.astype(E4NP)
        lo = (v - hi.astype(np.float32)).astype(E4NP)
        return hi, lo

    def pack_w(v8):
        # [D, E] -> [128(p), NDP, 2(i), E]  (contiguous per partition)
        return np.ascontiguousarray(
            v8.reshape(NDP, 2, 128, E).transpose(2, 0, 1, 3))

    def pack_x(v8, s):
        # [D, s] -> [s/512(sc), 128(p), NDP, 2(i), 512]
        return np.ascontiguousarray(
            v8.reshape(NDP, 2, 128, s // 512, 512).transpose(3, 2, 0, 1, 4))

    eye = np.eye(128, dtype=np.float32)
    mask = np.triu(np.ones((128, 128), dtype=np.float32)).astype(BFNP)
    ones16 = np.full((128, 1), 1.0 / 16.0, dtype=np.float32).astype(BFNP)

    in_maps = []
    for core in range(8):
        b, g = divmod(core, 4)
        hs = slice(4 * g, 4 * g + HL)
        xT = np.ascontiguousarray(x[b, :s].T) * 16.0
        xh, xl = hilo(xT)
        m = {"xh8": pack_x(xh, s), "xl8": pack_x(xl, s),
             "eye": eye, "mask": mask, "ones16": ones16}
        for wi, wn in enumerate(("q", "k", "v")):
            wT = w[hs, wi].transpose(2, 0, 1).reshape(D, E) * 256.0
            wh, wl = hilo(wT)
            m[f"w{wn}h8"] = pack_w(wh)
            m[f"w{wn}l8"] = pack_w(wl)
        woT = w_out[:, 4 * g * DH:(4 * g + HL) * DH].T * 256.0  # [E, D]
        woh, wol = hilo(woT)
        # [E, D] -> [128(p), 2(tp), 2(i), D]
        m["woh8"] = np.ascontiguousarray(
            woh.reshape(2, 2, 128, D).transpose(2, 0, 1, 3))
        m["wol8"] = np.ascontiguousarray(
            wol.reshape(2, 2, 128, D).transpose(2, 0, 1, 3))
        in_maps.append(m)
    return in_maps


_prog_cache = {}


def get_program(s=S):
    if s not in _prog_cache:
        _prog_cache[s] = build_program(s)
    return _prog_cache[s]


def kernel(x, w_in, w_out):
    nc = get_program(S)
    in_maps = shard_inputs(x, w_in, w_out)
    res = run_bass_kernel_spmd(nc, in_maps, core_ids=list(range(8)))
    out = np.empty((B, S, D), dtype=np.float32)
    for b in range(B):
        acc = np.zeros((S, D), dtype=np.float64)
        for g in range(4):
            acc += res.results[4 * b + g]["out_part"]
        out[b] = (acc * DESCALE).astype(np.float32)
    return out


if __name__ == "__main__":
    import reference

    inputs = reference.setup_inputs()
    out = kernel(**{k: np.asarray(v) for k, v in inputs.items()})
    print("kernel output:", out.shape, out.dtype)


# revision 36
# speedup vs baseline: 1.0696x; 1.0219x over previous
"""Causal multi-head attention (B=2, S=2048, D=2048, H=16, DH=128) on 8 TRN2
NeuronCores.

Sharding: data-parallel over batch (2) x tensor-parallel over heads (4 groups
of 4 heads). Core c handles batch c//4, heads 4*(c%4) .. 4*(c%4)+3. Each core
computes its heads' attention and a partial output projection; the host sums
the 4 partials per batch (the "all-reduce") and applies the 2^-12 descale.

Numerics / speed scheme (validated to rel err ~1.7e-3 in emulation):
  - QKV projection: 3-term fp8e4 hi/lo DoubleRow matmuls (K=256 per instr,
    0.5 cycles/row -> 0.75x the f32r cycle count). Host pre-quantizes
    16*x and 256*w into packed pair layouts [D/256, 128, 2, *]; the 2^-12
    descale is applied at the PSUM->SBUF copy. Q,K,V stored bf16.
  - scores: bf16 matmuls (1 cycle/row, no N>=256 floor so diagonal blocks
    trim at 128 granularity), f32 PSUM.
  - softmax: exp on ACT with scale 1/sqrt(dh), bias -10 (scores bounded),
    es in bf16. Causal masking: 128-wide triangular mask multiply on the
    first valid 128 columns of diagonal blocks only.
  - denominators: es-as-stationary matmuls (out [128q, 1] per 128-q chunk,
    ~free on the PE), transposed back to a [1, 512] strip via 4 single-row
    PE transposes, reciprocal on DVE, gpsimd partition broadcast. The ones
    vector is 1/16 so bc = 16/den and O is produced at 16x scale for fp8.
  - PV: bf16 matmuls accumulating in PSUM.
  - output projection: 3-term fp8e4 hi/lo DoubleRow over E-pairs; O
    quantized to fp8 hi/lo pair tiles at 16x scale on DVE; host divides
    the summed partials by 4096 (= 16*256).
Phases are interleaved: attention for q-chunk qc runs right after the
projection chunk sc=qc (it needs exactly K/V chunks 0..qc), and the output
projection for qc-1 is spread through the attention of qc to fill PE stalls.
"""

import sys

if "/opt/trn_rl_repo" not in sys.path:
    sys.path.insert(0, "/opt/trn_rl_repo")

import numpy as np
import ml_dtypes

import concourse.bass as bass  # noqa: F401  (registers AP types)
import concourse.tile as tile
from concourse import bacc, mybir
from concourse.bass_utils import run_bass_kernel_spmd

B, S, D = 2, 2048, 2048
H, DH = 16, 128
HL = H // 4          # heads per core
E = HL * DH          # local feature width (512)
SCALE = 1.0 / np.sqrt(DH)
CBIAS = 10.0         # > max causal score (8.70 measured on the real inputs)

F32 = mybir.dt.float32
F32R = mybir.dt.float32r
F8 = mybir.dt.float8e4
BF = mybir.dt.bfloat16
DR = mybir.MatmulPerfMode.DoubleRow
E4NP = ml_dtypes.float8_e4m3
BFNP = ml_dtypes.bfloat16

NSC = S // 512       # s-chunks of 512
NDP = D // 256       # D-pair groups (8)
DESCALE = 2.0 ** -12  # 1/(16*256)
WARMUP = 8
_EARLY = 3          # PE warm-up matmuls bridging the initial DMA window


def build_program(s=S):
    nsc = s // 512
    nc = bacc.Bacc("TRN2", target_bir_lowering=False, debug=False, num_devices=8)

    # dram layouts mirror the packed SBUF tiles exactly (contiguous per
    # partition), so each tensor loads with a single 2dim-collapsible DMA
    xh8 = nc.dram_tensor("xh8", [s // 512, 128, NDP, 2, 512], F8,
                         kind="ExternalInput").ap()
    xl8 = nc.dram_tensor("xl8", [s // 512, 128, NDP, 2, 512], F8,
                         kind="ExternalInput").ap()
    w8 = {}
    for wn in ("q", "k", "v"):
        for part in ("h", "l"):
            name = f"w{wn}{part}8"
            w8[wn, part] = nc.dram_tensor(name, [128, NDP, 2, E], F8,
                                          kind="ExternalInput").ap()
    woh8 = nc.dram_tensor("woh8", [128, 2, 2, D], F8, kind="ExternalInput").ap()
    wol8 = nc.dram_tensor("wol8", [128, 2, 2, D], F8, kind="ExternalInput").ap()
    eye = nc.dram_tensor("eye", [128, 128], F32R, kind="ExternalInput").ap()
    mask = nc.dram_tensor("mask", [128, 128], BF, kind="ExternalInput").ap()
    ones16 = nc.dram_tensor("ones16", [128, 1], BF, kind="ExternalInput").ap()
    out_part = nc.dram_tensor("out_part", [s, D], BF, kind="ExternalOutput").ap()

    with tile.TileContext(nc) as tc:
        _emit(tc, nc, xh8, xl8, w8, woh8, wol8, eye, mask, ones16, out_part, nsc)
    nc.compile()
    return nc


def _emit(tc, nc, xh8, xl8, w8, woh8, wol8, eye, mask, ones16, out_part, nsc):
    from contextlib import ExitStack
    ctx = ExitStack()
    s = nsc * 512

    # ---- constants / long-lived tiles -----------------------------------
    const_pool = ctx.enter_context(tc.tile_pool(name="const", bufs=1))
    bias_t = const_pool.tile([128, 1], F32, tag="bias", name="bias")
    nc.vector.memset(bias_t[:], -CBIAS)
    eye_t = const_pool.tile([128, 128], F32R, tag="eye", name="eye")
    mask_t = const_pool.tile([128, 128], BF, tag="mask", name="mask")
    ones_t = const_pool.tile([128, 1], BF, tag="ones", name="ones")

    # ---- PE warm-up during the initial DMA ramp -------------------------
    with tc.tile_pool(name="warm", bufs=1) as warm_pool, \
         tc.tile_pool(name="warmps", bufs=1, space="PSUM") as warm_ps:
        wsrc = warm_pool.tile([128, 512], F32, tag="wsrc", name="wsrc")
        nc.vector.memset(wsrc[:], 0.0)
        wps = warm_ps.tile([128, 512], F32, tag="wps", name="wps")
        for _ in range(WARMUP):
            nc.tensor.matmul(wps[:, :256], wsrc[:, :128], wsrc[:, :256],
                             start=True, stop=True)

    # ---- persistent products --------------------------------------------
    qk_pool = ctx.enter_context(tc.tile_pool(name="qk", bufs=1))
    QT = [qk_pool.tile([128, s], BF, tag=f"qT{h}", name=f"qT{h}") for h in range(HL)]
    KT = [qk_pool.tile([128, s], BF, tag=f"kT{h}", name=f"kT{h}") for h in range(HL)]
    v_pool = ctx.enter_context(tc.tile_pool(name="v", bufs=1))
    V = [v_pool.tile([128, E], BF, tag=f"v{kt}", name=f"v{kt}")
         for kt in range(4 * nsc)]

    # ---- weights (one batched tile + one DMA per tensor: the HWDGE
    # descriptor-generation slot costs ~625ns per dma_start, so many small
    # tile DMAs serialize the startup) ------------------------------------
    w_pool = ctx.enter_context(tc.tile_pool(name="w", bufs=1))
    wt = {}
    for wn in ("q", "k", "v"):
        for part in ("h", "l"):
            wt[wn, part] = w_pool.tile([128, NDP, 2, E], F8,
                                       tag=f"w{wn}{part}", name=f"w{wn}{part}")
    wo_pool = ctx.enter_context(tc.tile_pool(name="wo", bufs=1))
    wo_t = {}
    for part in ("h", "l"):
        wo_t[part] = wo_pool.tile([128, 2, 2, D], F8, tag=f"wo{part}",
                                  name=f"wo{part}")

    # O pair tiles (fp8 hi/lo, 16x scale), per head-pair t, double-buffered
    # across qc (outproj of qc-1 runs during attention of qc).
    o_pool = ctx.enter_context(tc.tile_pool(name="o", bufs=2))

    # ---- phase-2 pools ---------------------------------------------------
    es_pool = ctx.enter_context(tc.tile_pool(name="es", bufs=6))
    nrm_pool = ctx.enter_context(tc.tile_pool(name="nrm", bufs=2))
    res_pool = ctx.enter_context(tc.tile_pool(name="res", bufs=4))
    x_pool = ctx.enter_context(tc.tile_pool(name="x", bufs=2))

    pa = ctx.enter_context(tc.tile_pool(name="pa", bufs=2, space="PSUM"))
    sp = ctx.enter_context(tc.tile_pool(name="sp", bufs=2, space="PSUM"))
    op = ctx.enter_context(tc.tile_pool(name="op", bufs=2, space="PSUM"))
    dnp = ctx.enter_context(tc.tile_pool(name="dnp", bufs=1, space="PSUM"))
    ptp = ctx.enter_context(tc.tile_pool(name="ptp", bufs=1, space="PSUM"))

    # ---- initial DMAs (batched, priority ordered) ------------------------
    xdram = {"h": xh8, "l": xl8}

    def load_x_chunk(sc):
        xt = {}
        for part in ("h", "l"):
            t = x_pool.tile([128, NDP, 2, 512], F8, tag=f"x{part}",
                            name=f"x{part}")
            nc.sync.dma_start(t[:], xdram[part][sc])
            xt[part] = t
        return xt

    x_cur = {}
    nc.sync.dma_start(wt["q", "h"][:], w8["q", "h"])
    t = x_pool.tile([128, NDP, 2, 512], F8, tag="xh", name="xh")
    nc.sync.dma_start(t[:], xh8[0])
    x_cur["h"] = t
    nc.sync.dma_start(wt["q", "l"][:], w8["q", "l"])
    t = x_pool.tile([128, NDP, 2, 512], F8, tag="xl", name="xl")
    nc.sync.dma_start(t[:], xl8[0])
    x_cur["l"] = t
    nc.sync.dma_start(wt["k", "h"][:], w8["k", "h"])
    nc.sync.dma_start(wt["k", "l"][:], w8["k", "l"])
    nc.sync.dma_start(wt["v", "h"][:], w8["v", "h"])
    nc.sync.dma_start(wt["v", "l"][:], w8["v", "l"])
    nc.sync.dma_start(eye_t[:], eye)
    nc.sync.dma_start(mask_t[:], mask)
    nc.sync.dma_start(ones_t[:], ones16)
    nc.sync.dma_start(wo_t["h"][:], woh8)
    nc.sync.dma_start(wo_t["l"][:], wol8)

    # pending output-projection units: (qc, j, dc) tuples
    pending_units = []
    o_tiles = {}   # qc -> {("h"|"l", t): tile}

    res_cur = [None]

    def emit_unit():
        if not pending_units:
            return
        qc, j, dc = pending_units.pop(0)
        ot = o_tiles[qc]
        jsl = slice(j * 128, (j + 1) * 128)
        dsl = slice(dc * 512, (dc + 1) * 512)
        ps_f = pa.tile([128, 512], F32, tag="pa", name="pf")
        steps = []
        for t in range(2):
            steps += [(ot["h", t], wo_t["h"], t), (ot["h", t], wo_t["l"], t),
                      (ot["l", t], wo_t["h"], t)]
        for i, (o8, w8t, t) in enumerate(steps):
            nc.tensor.matmul(ps_f[:], o8[:, :, jsl], w8t[:, t, :, dsl],
                             start=(i == 0), stop=(i == len(steps) - 1),
                             perf_mode=DR)
        # batch the 4 d-chunks of one 128-row block into a single out DMA
        if dc == 0:
            res_cur[0] = res_pool.tile([128, D], BF, tag="res", name="res")
        nc.vector.tensor_copy(res_cur[0][:, dsl], ps_f[:])
        if dc == 3:
            nc.sync.dma_start(
                out_part[qc * 512 + j * 128:qc * 512 + (j + 1) * 128, :],
                res_cur[0][:])

    def ph1_quanta(sc, xt):
        """Phase-1 chunk sc as a generator of ~8-matmul quanta. Chains are
        emitted in interleaved PAIRS at term-group granularity (2 open PSUM
        chains = pa bufs), so a chain stalled on a late-arriving DMA tile
        (w_lo / x_lo) never blocks the partner chain's ready work — matters
        for the DMA-paced first chunk."""
        ssl = slice(sc * 512, (sc + 1) * 512)

        def qk_chain(wn, h):
            hsl = slice(h * 128, (h + 1) * 128)
            ps = pa.tile([128, 512], F32, tag="pa", name="pqk")
            step = 0
            for part_w, part_x in (("h", "h"), ("l", "h"), ("h", "l")):
                for dp in range(NDP):
                    nc.tensor.matmul(ps[:], wt[wn, part_w][:, dp, :, hsl],
                                     xt[part_x][:, dp],
                                     start=(step == 0),
                                     stop=(step == 3 * NDP - 1),
                                     perf_mode=DR)
                    step += 1
                yield
            if wn == "q":
                nc.scalar.mul(QT[h][:, ssl], ps[:], DESCALE)
            else:
                nc.vector.tensor_scalar_mul(KT[h][:, ssl], ps[:], DESCALE)

        def v_chain(j):
            kt = sc * 4 + j
            jsl = slice(j * 128, (j + 1) * 128)
            ps_v = pa.tile([128, E], F32, tag="pa", name="pv")
            step = 0
            for part_x, part_w in (("h", "h"), ("h", "l"), ("l", "h")):
                for dp in range(NDP):
                    nc.tensor.matmul(ps_v[:], xt[part_x][:, dp, :, jsl],
                                     wt["v", part_w][:, dp],
                                     start=(step == 0),
                                     stop=(step == 3 * NDP - 1),
                                     perf_mode=DR)
                    step += 1
                yield
            nc.vector.tensor_scalar_mul(V[kt][:], ps_v[:], DESCALE)

        chains = [qk_chain("q", h) for h in range(HL)]
        chains += [qk_chain("k", h) for h in range(HL)]
        chains += [v_chain(j) for j in range(4)]
        for a, b in zip(chains[0::2], chains[1::2]):
            for ga, gb in zip(a, b):
                yield
                yield
            for _ in a:
                yield
            for _ in b:
                yield

    NQUANTA = HL * 2 * 3 + 4 * 3  # 36 quanta per phase-1 chunk
    EARLY = _EARLY                     # heads of att(qc) pulled into iteration qc

    def get_o_tiles(qc):
        if qc not in o_tiles:
            ot = {}
            for t in range(2):
                for part in ("h", "l"):
                    ot[part, t] = o_pool.tile([128, 2, 512], F8,
                                              tag=f"o{part}{t}",
                                              name=f"o{part}{t}")
            o_tiles[qc] = ot
        return o_tiles[qc]

    def att_head(qc, h, block_cb):
        """Attention for (head h, q-chunk qc); block_cb() paces filler work
        (phase-1 quanta / outproj units) after each k-block."""
        ot = get_o_tiles(qc)
        nkb = 4 * (qc + 1)
        hsl = slice(h * 128, (h + 1) * 128)
        ps_o = op.tile([128, 512], F32, tag="po", name="po")
        pden = dnp.tile([128, 4], F32, tag="pden", name="pden")
        # PSUM zeroing is 2KB-bank granular, so the four interleaved
        # per-column accumulation groups must not use start=True: memset
        # the bank once and accumulate with start=False.
        nc.vector.memset(pden[:], 0.0)
        for kb in range(nkb):
            kbloc = kb - 4 * qc
            s0 = max(0, kbloc * 128)
            ps_s = sp.tile([128, 512], F32, tag="ps", name="ps")
            nc.tensor.matmul(
                ps_s[:, s0:], KT[h][:, kb * 128:(kb + 1) * 128],
                QT[h][:, qc * 512 + s0:(qc + 1) * 512],
                start=True, stop=True)
            es = es_pool.tile([128, 512], BF, tag="es", name="es")
            nc.scalar.activation(es[:, s0:], ps_s[:, s0:],
                                 mybir.ActivationFunctionType.Exp,
                                 bias=bias_t[:], scale=float(SCALE))
            if kbloc >= 0:
                nc.vector.tensor_mul(es[:, s0:s0 + 128], es[:, s0:s0 + 128],
                                     mask_t[:])
            nc.tensor.matmul(ps_o[:, s0:], V[kb][:, hsl], es[:, s0:],
                             start=(kb == 0), stop=(kb == nkb - 1))
            for j in range(max(0, kbloc), 4):
                nc.tensor.matmul(pden[:, j:j + 1],
                                 es[:, j * 128:(j + 1) * 128], ones_t[:],
                                 start=False,
                                 stop=(kb == 4 * qc + j),
                                 skip_group_check=True)
            block_cb()
        # normalization: den [128q,4] -> [1,512] strip -> recip -> bc
        # (f32r: same bits as f32, 1.5 instead of 2.0 transpose cycles/row)
        den_sb = nrm_pool.tile([128, 4], F32R, tag="den", name="den")
        nc.vector.tensor_copy(den_sb[:], pden[:])
        pt = ptp.tile([1, 512], F32R, tag="pt", name="pt")
        nc.vector.memset(pt[:].bitcast(F32), 0.0)
        for j in range(4):
            nc.tensor.matmul(pt[0:1, j * 128:(j + 1) * 128],
                             den_sb[:, j:j + 1], eye_t[:],
                             start=False, stop=True, is_transpose=True,
                             skip_group_check=True)
        recip = nrm_pool.tile([1, 512], F32, tag="recip", name="recip")
        nc.vector.reciprocal(recip[:], pt[:])
        bc = nrm_pool.tile([128, 512], F32, tag="bc", name="bc")
        nc.gpsimd.partition_broadcast(bc[:], recip[0:1, :])
        of = nrm_pool.tile([128, 512], F32, tag="of", name="of")
        nc.vector.tensor_mul(of[:], ps_o[:], bc[:])
        t, i = h // 2, h % 2
        nc.vector.tensor_copy(ot["h", t][:, i, :], of[:])
        nc.vector.tensor_sub(ot["l", t][:, i, :], of[:], ot["h", t][:, i, :])

    # ---- main interleaved loop ------------------------------------------
    # iteration it: heads EARLY..4 of att(qc=it-1), phase-1 chunk sc=it
    # spread through them, then heads 0..EARLY of att(qc=it) right after the
    # phase-1 flush (pulling exp/ACT load out of the tail iteration).
    for it in range(nsc + 1):
        sc = it if it < nsc else None
        qc = it - 1
        gen = None
        if sc is not None:
            xt = x_cur if sc == 0 else load_x_chunk(sc)
            gen = ph1_quanta(sc, xt)

        main_heads = [] if qc < 0 else [(qc, h) for h in range(
            0 if qc == 0 else EARLY, HL)]
        early_heads = [] if sc is None else [(sc, h) for h in range(EARLY)]
        nblocks = (sum(4 * (q + 1) for q, _ in main_heads)
                   + sum(4 * (q + 1) for q, _ in early_heads))
        state = {"blk": 0, "q": 0, "u": 0}
        n_units = len(pending_units)

        def block_cb():
            state["blk"] += 1
            if gen is not None:
                while state["q"] * nblocks < NQUANTA * state["blk"]:
                    if next(gen, None) is None:
                        break
                    state["q"] += 1
            while state["u"] * nblocks < n_units * state["blk"]:
                emit_unit()
                state["u"] += 1

        for q, h in main_heads:
            att_head(q, h, block_cb)
        if gen is not None:
            for _ in gen:
                pass
        for q, h in early_heads:
            att_head(q, h, block_cb)
        while pending_units:
            emit_unit()
        if qc >= 0:
            pending_units += [(qc, j, dc) for j in range(4) for dc in range(4)]
        if qc == nsc - 1:
            while pending_units:
                emit_unit()
    ctx.close()


def shard_inputs(x, w_in, w_out, s=S):
    """Return the 8 per-core input dicts (host-side fp8 hi/lo packing)."""
    x = np.asarray(x, dtype=np.float32)
    w = np.asarray(w_in, dtype=np.float32).reshape(H, 3, DH, D)
    w_out = np.asarray(w_out, dtype=np.float32)

    def hilo(v):
        hi = v.astype(E4NP)
        lo = (v - hi.astype(np.float32)).astype(E4NP)
        return hi, lo

    def pack_w(v8):
        # [D, E] -> [128(p), NDP, 2(i), E]  (contiguous per partition)
        return np.ascontiguousarray(
            v8.reshape(NDP, 2, 128, E).transpose(2, 0, 1, 3))

    def pack_x(v8, s):
        # [D, s] -> [s/512(sc), 128(p), NDP, 2(i), 512]
        return np.ascontiguousarray(
            v8.reshape(NDP, 2, 128, s // 512, 512).transpose(3, 2, 0, 1, 4))

    eye = np.eye(128, dtype=np.float32)
    mask = np.triu(np.ones((128, 128), dtype=np.float32)).astype(BFNP)
    ones16 = np.full((128, 1), 1.0 / 16.0, dtype=np.float32).astype(BFNP)

    in_maps = []
    for core in range(8):
        b, g = divmod(core, 4)
        hs = slice(4 * g, 4 * g + HL)
        xT = np.ascontiguousarray(x[b, :s].T) * 16.0
        xh, xl = hilo(xT)
        m = {"xh8": pack_x(xh, s), "xl8": pack_x(xl, s),
             "eye": eye, "mask": mask, "ones16": ones16}
        for wi, wn in enumerate(("q", "k", "v")):
            wT = w[hs, wi].transpose(2, 0, 1).reshape(D, E) * 256.0
            wh, wl = hilo(wT)
            m[f"w{wn}h8"] = pack_w(wh)
            m[f"w{wn}l8"] = pack_w(wl)
        woT = w_out[:, 4 * g * DH:(4 * g + HL) * DH].T * 256.0  # [E, D]
        woh, wol = hilo(woT)
        # [E, D] -> [128(p), 2(tp), 2(i), D]
        m["woh8"] = np.ascontiguousarray(
            woh.reshape(2, 2, 128, D).transpose(2, 0, 1, 3))
        m["wol8"] = np.ascontiguousarray(
            wol.reshape(2, 2, 128, D).transpose(2, 0, 1, 3))
        in_maps.append(m)
    return in_maps


_prog_cache = {}


def get_program(s=S):
    if s not in _prog_cache:
        _prog_cache[s] = build_program(s)
    return _prog_cache[s]


def kernel(x, w_in, w_out):
    nc = get_program(S)
    in_maps = shard_inputs(x, w_in, w_out)
    res = run_bass_kernel_spmd(nc, in_maps, core_ids=list(range(8)))
    out = np.empty((B, S, D), dtype=np.float32)
    for b in range(B):
        acc = np.zeros((S, D), dtype=np.float64)
        for g in range(4):
            acc += res.results[4 * b + g]["out_part"]
        out[b] = (acc * DESCALE).astype(np.float32)
    return out


if __name__ == "__main__":
    import reference

    inputs = reference.setup_inputs()
    out = kernel(**{k: np.asarray(v) for k, v in inputs.items()})
    print("kernel output:", out.shape, out.dtype)
